# revision 40
# baseline (speedup 1.0000x reference)
"""Multi-head attention (SEQ=4096, d_model=1024, 16 heads of d=64) on 8 TRN2
NeuronCores, tensor-parallel over heads (2 heads/core), per-head AllToAll to
sequence-parallel before the output projection.

v3 design (vs 480us baseline / 652us v2):
  - Block-outer attention sweeps with JIT projections: q/k/v are projected
    per column-group as their DMA chunks land, inside sweep 1, hiding the
    24MB input load and the projection PE work under attention.
  - AV matmuls run fp8 DoubleRow (K=256: two key blocks per matmul):
    vh2 [ks, pair, 2, 80] e4m3 (col 64 = ones -> softmax denominator),
    pt [ks, 2, qs] e5m2.  exp(s/8-0.5) alternates ScalarE LUT (e5m2 out)
    and VectorE Schraudolph uint8 bits = 0.72135*s + 50.62.
  - Batched normalization: the 4 sums rows of a sweep land at partitions
    {0,32,64,96} of one tile, ONE reciprocal, two selector-matmul
    broadcasts, DVE muls read the broadcast from PSUM.  The whole
    normalize tail is hooked into the NEXT sweep's PE stream so the PE
    never idles on the reciprocal latency (the v2 mistake: 16 single-
    partition reciprocals at 4us each on sweep boundaries).
  - Score pipeline depth 4 in sweeps 2-4 (sco tiles alternate two PSUM
    pools); FC head-0 passes fill the sweep3->4 boundary.
  - Outputs ship pre-normalized: one AllToAll per head, no sums A2A.
"""

import os
import sys

sys.path.insert(0, "/opt/trn_rl_repo")

import numpy as np
import ml_dtypes

import concourse.bass as bass
import concourse.mybir as mybir
import concourse.tile as tile
from concourse import bacc
from concourse.bass_utils import run_bass_kernel_spmd

SEQ = 4096
DM = 1024
NH = 16
DK = 64
DV = 64
CORES = 8
P = 128
HL = 2 * DK  # 128: two heads' head-dim per core
SROWS = SEQ // CORES  # 512 output rows per core
MO = DM // P  # 8 m-chunks of d_model
F32 = mybir.dt.float32
BF16 = mybir.dt.bfloat16
FP8E4 = mybir.dt.float8e4
FP8E5 = mybir.dt.float8e5
U8 = mybir.dt.uint8

AV_MODE = os.environ.get("AV_MODE", "fp8")  # fp8 (DoubleRow) | bf16 fallback

# exp(x/8 - SHIFT).  Real scores span +-75 (heavy tails): e5m2 Schraudolph
# bits = 0.72135*x + E5_B must stay in (0, 124) -> SHIFT=0.5: bits [2.9, 111].
ESHIFT = 0.5
E5_A = 4.0 / (8.0 * np.log(2.0))
E5_B = 4.0 * (15.0 - ESHIFT / np.log(2.0)) - 0.172
BF_A = 128.0 / (8.0 * np.log(2.0))
BF_B = 16256.0 - 5.5 - 128.0 * ESHIFT / np.log(2.0)


def build():
    kb = SEQ // P  # 32 key blocks
    npair = kb // 2  # 16 DoubleRow pairs
    qcw = 512
    sb_blocks = SROWS // P  # 4
    hchunks = (CORES * DK) // P  # 4 FC lhsT chunks per head
    fp8 = AV_MODE == "fp8"

    nc = bacc.Bacc(
        "TRN2",
        target_bir_lowering=False,
        debug=False,
        enable_asserts=True,
        num_devices=CORES,
    )

    qT = nc.dram_tensor("qT", [DM, SEQ], BF16, kind="ExternalInput").ap()
    kT = nc.dram_tensor("kT", [DM, SEQ], BF16, kind="ExternalInput").ap()
    vT = nc.dram_tensor("vT", [DM, SEQ], BF16, kind="ExternalInput").ap()
    wqT = nc.dram_tensor("wqT", [DM, HL], BF16, kind="ExternalInput").ap()
    wkT = nc.dram_tensor("wkT", [DM, HL], BF16, kind="ExternalInput").ap()
    wvT = nc.dram_tensor("wvT", [DM, HL], BF16, kind="ExternalInput").ap()
    # pre-permuted on host to match the post-A2A dv row order; fp8 e4m3
    # scaled x32 (outputs ship x16; the epilogue relu divides by 512)
    wfcT = nc.dram_tensor("wfcT", [DM, DM], FP8E4, kind="ExternalInput").ap()
    qres = nc.dram_tensor("qres", [SROWS, DM], F32, kind="ExternalInput").ap()
    # selector for the reciprocal broadcast (partition-1 writes are not
    # expressible on-device): selb[p, pair*128+sub*64+m] = (p==32*(2*pair+sub))
    selb_in = nc.dram_tensor("selb", [P, 2 * P], BF16, kind="ExternalInput").ap()
    out = nc.dram_tensor("out", [SROWS, DM], F32, kind="ExternalOutput").ap()

    qT_r = qT.rearrange("(o p) s -> p o s", p=P)
    kT_r = kT.rearrange("(o p) s -> p o s", p=P)
    vT_r = vT.rearrange("(o p) s -> p o s", p=P)
    wqT_r = wqT.rearrange("(o p) h -> p o h", p=P)
    wkT_r = wkT.rearrange("(o p) h -> p o h", p=P)
    wvT_r = wvT.rearrange("(o p) h -> p o h", p=P)
    # row (2*pp+s)*128+p -> [p, pp, s, d] for DoubleRow chunk pairs
    wfcT_r = wfcT.rearrange("(a s p) d -> p a s d", p=P, s=2)
    qres_r = qres.rearrange("(b p) d -> p b d", p=P)
    out_r = out.rearrange("(b p) d -> p b d", p=P)

    with tile.TileContext(nc) as tc:
        with (
            tc.tile_pool(name="const", bufs=1) as cpool,
            tc.tile_pool(name="xq", bufs=4) as xqpool,
            tc.tile_pool(name="xk", bufs=3) as xkpool,
            tc.tile_pool(name="xv", bufs=3) as xvpool,
            tc.tile_pool(name="pt", bufs=8) as ptpool,
            tc.tile_pool(name="avsb", bufs=2) as avsbpool,
            tc.tile_pool(name="eo", bufs=3) as eopool,
            tc.tile_pool(name="avps", bufs=4, space="PSUM") as avps,
            tc.tile_pool(name="scps", bufs=2, space="PSUM") as scps,
            tc.tile_pool(name="pjps", bufs=2, space="PSUM") as pjps,
            tc.tile_pool(name="dram", bufs=1, space="DRAM") as dr,
        ):
            # ---- first q chunks + weights (startup-critical order) ----
            xq = {}

            def load_xq(g, eng):
                t = xqpool.tile([P, MO, qcw], BF16, tag="xq", name=f"xq{g}")
                eng.dma_start(t[:], qT_r[:, :, g * qcw : (g + 1) * qcw])
                xq[g] = t

            wq_sb = cpool.tile([P, MO, HL], BF16, tag="wq")
            wk_sb = cpool.tile([P, MO, HL], BF16, tag="wk")
            wv_sb = cpool.tile([P, MO, HL], BF16, tag="wv")
            selb = cpool.tile([P, 2 * P], BF16, tag="selb")
            xks, xvs = {}, {}

            def load_kv(bg):
                xks[bg] = xkpool.tile([P, MO, qcw], BF16, tag="xk",
                                      name=f"xk{bg}")
                eng = nc.sync if bg % 2 == 0 else nc.gpsimd
                eng.dma_start(xks[bg][:], kT_r[:, :, bg * qcw : (bg + 1) * qcw])
                xvs[bg] = xvpool.tile([P, MO, qcw], BF16, tag="xv",
                                      name=f"xv{bg}")
                eng2 = nc.gpsimd if bg % 2 == 0 else nc.sync
                eng2.dma_start(xvs[bg][:], vT_r[:, :, bg * qcw : (bg + 1) * qcw])

            load_xq(0, nc.sync)
            nc.sync.dma_start(wq_sb[:], wqT_r[:])
            load_xq(1, nc.gpsimd)
            nc.gpsimd.dma_start(wk_sb[:], wkT_r[:])
            load_xq(2, nc.sync)
            load_xq(3, nc.gpsimd)
            nc.gpsimd.dma_start(wv_sb[:], wvT_r[:])
            nc.sync.dma_start(selb[:], selb_in[:])
            load_kv(0)
            load_kv(1)

            qhT2 = cpool.tile([P, SEQ], BF16, tag="qhT2")
            khT2z = [
                cpool.tile([P, SEQ], BF16, tag=f"khT2z{h}", name=f"khT2z{h}")
                for h in range(2)
            ]
            # ~1e-30 not 0.0: zero weights throttle the PE (activity gating)
            nc.vector.memset(khT2z[0][DK:P, :], 1e-30)
            nc.vector.memset(khT2z[1][0:DK, :], 1e-30)
            if fp8:
                # [ks, pair, slot, 80] e4m3; col 64 = ones (softmax denom),
                # cols 65-79 pad to a 16B DoubleRow weight stride
                vh2 = [
                    cpool.tile([P, npair, 2, 80], FP8E4, tag=f"vh{h}", name=f"vh{h}")
                    for h in range(2)
                ]
                for h in range(2):
                    nc.vector.memset(vh2[h][:, :, :, DV : DV + 1], 1.0)
                    nc.vector.memset(vh2[h][:, :, :, DV + 1 :], 0.002)
            else:
                vh2 = [
                    cpool.tile([P, kb, DV + 1], BF16, tag=f"vh{h}", name=f"vh{h}")
                    for h in range(2)
                ]
                for h in range(2):
                    nc.vector.memset(vh2[h][:, :, DV : DV + 1], 1.0)

            # normalized outputs (x16, fp8): qc j slab at partitions (j%2)*64,
            # cols j*512 (per head) -- ship DMAs slice per destination
            outT2 = [
                cpool.tile([P, SEQ], FP8E4, tag=f"outT2{h}", name=f"outT2{h}")
                for h in range(2)
            ]
            # sweep's 4 sums rows at partitions {0,32,64,96}; bf16 so the
            # selector-matmul broadcast runs as a fast bf16 matmul
            srow = cpool.tile([P, qcw], BF16, tag="srow")
            nc.vector.memset(srow[:], 1.0)
            negshift = cpool.tile([P, 1], F32, tag="negshift")
            nc.vector.memset(negshift[:], -float(ESHIFT))

            # ---- projection helpers ----
            def qproj_group(g):
                qp = pjps.tile([P, qcw], F32, tag="pj", name=f"qp{g}")
                for o in range(MO):
                    nc.tensor.matmul(
                        qp[:],
                        wq_sb[:, o, :],
                        xq[g][:, o, :],
                        start=(o == 0),
                        stop=(o == MO - 1),
                    )
                nc.scalar.copy(out=qhT2[:, g * qcw : (g + 1) * qcw], in_=qp[:])

            def kproj_group(bg, xk):
                kp = pjps.tile([P, qcw], F32, tag="pj", name=f"kp{bg}")
                for o in range(MO):
                    nc.tensor.matmul(
                        kp[:],
                        wk_sb[:, o, :],
                        xk[:, o, :],
                        start=(o == 0),
                        stop=(o == MO - 1),
                    )
                c0 = bg * qcw
                nc.scalar.copy(out=khT2z[0][0:DK, c0 : c0 + qcw], in_=kp[0:DK])
                nc.scalar.copy(out=khT2z[1][DK:HL, c0 : c0 + qcw], in_=kp[DK:HL])

            def vproj_block(b, xv, boff):
                pv = pjps.tile([P, HL], F32, tag="pj", name=f"pv{b}")
                for o in range(MO):
                    nc.tensor.matmul(
                        pv[:],
                        xv[:, o, boff : boff + P],
                        wv_sb[:, o, :],
                        start=(o == 0),
                        stop=(o == MO - 1),
                    )
                if fp8:
                    nc.vector.tensor_copy(
                        out=vh2[0][:, b // 2, b % 2, :DV], in_=pv[:, :DK]
                    )
                    nc.vector.tensor_copy(
                        out=vh2[1][:, b // 2, b % 2, :DV], in_=pv[:, DK:HL]
                    )
                else:
                    nc.vector.tensor_copy(out=vh2[0][:, b, :DV], in_=pv[:, :DK])
                    nc.vector.tensor_copy(out=vh2[1][:, b, :DV], in_=pv[:, DK:HL])

            exp_ctr = [0]

            def exp_tile(dst, dslot, sco):
                use_dve = exp_ctr[0] % 2 == 0
                exp_ctr[0] += 1
                if fp8:
                    if use_dve:
                        nc.vector.tensor_scalar(
                            out=dst[:, dslot, :].bitcast(U8),
                            in0=sco[:],
                            scalar1=float(E5_A),
                            scalar2=float(E5_B),
                            op0=mybir.AluOpType.mult,
                            op1=mybir.AluOpType.add,
                        )
                    else:
                        nc.scalar.activation(
                            out=dst[:, dslot, :],
                            in_=sco[:],
                            func=mybir.ActivationFunctionType.Exp,
                            scale=0.125,
                            bias=negshift[:],
                        )
                else:
                    if use_dve:
                        nc.vector.tensor_scalar(
                            out=dst[:].bitcast(mybir.dt.int16),
                            in0=sco[:],
                            scalar1=float(BF_A),
                            scalar2=float(BF_B),
                            op0=mybir.AluOpType.mult,
                            op1=mybir.AluOpType.add,
                        )
                    else:
                        nc.scalar.activation(
                            out=dst[:],
                            in_=sco[:],
                            func=mybir.ActivationFunctionType.Exp,
                            scale=0.125,
                            bias=negshift[:],
                        )

            # ---- batched drain machinery ----
            # stage 1 (at sweep end): free the avT banks fast -- ACT copies
            # the 64 dv rows to an SBUF slab, DVE copies the sums row into
            # srow at partition 32*j.
            # stage 2 (hooked into the NEXT sweep's PE stream): one
            # reciprocal over srow, two selector-matmul broadcasts, DVE
            # muls write normalized bf16 into outT2.
            def drain_stage1(h, jbase, avts):
                avsb = avsbpool.tile([P, 2, qcw], BF16, tag="avsb",
                                     name=f"avsb{h}_{jbase}")
                for j in range(4):
                    nc.scalar.copy(
                        out=avsb[(j % 2) * DK : (j % 2) * DK + DK, j // 2, :],
                        in_=avts[j][:DV, :],
                    )
                    nc.vector.tensor_copy(
                        out=srow[32 * j : 32 * j + 1, :],
                        in_=avts[j][DV : DV + 1, :],
                    )
                return avsb

            def drain_stage2(h, jbase, avsb, split=False):
                if not split:
                    with nc.allow_low_precision(reason="softmax denominators"):
                        nc.vector.reciprocal(srow[:], srow[:])
                for pair in range(2):
                    if split:
                        with nc.allow_low_precision(reason="softmax denominators"):
                            nc.vector.reciprocal(
                                srow[pair * DK : (pair + 1) * DK, :],
                                srow[pair * DK : (pair + 1) * DK, :],
                            )
                    bcp = pjps.tile([P, qcw], F32, tag="pj",
                                    name=f"bcp{h}_{jbase}_{pair}")
                    nc.tensor.matmul(
                        bcp[:],
                        selb[:, pair * P : (pair + 1) * P],
                        srow[:],
                        start=True,
                        stop=True,
                    )
                    for sub in range(2):
                        j = 2 * pair + sub
                        jq = jbase + j
                        p0 = (j % 2) * DK
                        nc.vector.tensor_mul(
                            out=outT2[h][p0 : p0 + DK, jq * qcw : (jq + 1) * qcw],
                            in0=avsb[p0 : p0 + DK, j // 2, :],
                            in1=bcp[p0 : p0 + DK, :],
                        )
                        ship_qc(h, jq)

            # dvi ship DMAs, one per destination core, emitted as soon as
            # that qc's normalized slab exists
            dvi_t = [None, None]

            def ship_qc(h, jq):
                eng = nc.sync if jq % 2 == 0 else nc.gpsimd
                eng.dma_start(
                    dvi_t[h][jq * DK : (jq + 1) * DK, :],
                    outT2[h][(jq % 2) * DK : (jq % 2) * DK + DK,
                             jq * qcw : (jq + 1) * qcw],
                )

            # ---- attention sweep ----
            def sweep(h, jbase, with_proj, hooks=None):
                avts = [
                    avps.tile([80 if fp8 else DV + 1, qcw], F32, tag="av",
                              name=f"av{h}_{jbase + j}")
                    for j in range(4)
                ]
                pts = {}
                deep = not with_proj
                for b in range(kb):
                    if with_proj:
                        bg = b // 4
                        if b % 4 == 0:
                            if bg + 2 < kb // 4:
                                load_kv(bg + 2)
                            kproj_group(bg, xks[bg])
                        vproj_block(b, xvs[bg], (b % 4) * P)
                    for j in range(4):
                        jq = jbase + j
                        q0 = jq * qcw
                        # depth-4 score pipeline outside sweep 1
                        pool = pjps if (deep and (b * 4 + j) % 2 == 1) else scps
                        sco = pool.tile([P, qcw], F32, tag=pool is scps and "sc" or "pj",
                                        name=f"sc{b}_{j}")
                        nc.tensor.matmul(
                            sco[:],
                            khT2z[h][:, b * P : (b + 1) * P],
                            qhT2[:, q0 : q0 + qcw],
                            start=True,
                            stop=True,
                        )
                        if fp8:
                            if b % 2 == 0:
                                pts[j] = ptpool.tile([P, 2, qcw], FP8E5, tag="pt",
                                                     name=f"pt{b}_{j}")
                            exp_tile(pts[j], b % 2, sco)
                            if b % 2 == 1:
                                nc.tensor.matmul(
                                    avts[j][:],
                                    vh2[h][:, b // 2, :, :],
                                    pts[j][:],
                                    start=(b == 1),
                                    stop=(b == kb - 1),
                                    perf_mode=mybir.MatmulPerfMode.DoubleRow,
                                )
                        else:
                            ptb = ptpool.tile([P, qcw], BF16, tag="pt",
                                              name=f"pt{b}_{j}")
                            exp_tile(ptb, 0, sco)
                            nc.tensor.matmul(
                                avts[j][:],
                                vh2[h][:, b, :],
                                ptb[:],
                                start=(b == 0),
                                stop=(b == kb - 1),
                            )
                    if hooks and b in hooks:
                        hooks[b]()
                return drain_stage1(h, jbase, avts)

            # ---- AllToAll of normalized dv rows ----
            a2a_out = [None, None]

            def ship_head(h):
                dvo = dr.tile([CORES * DK, SROWS], FP8E4, name=f"a2advo{h}")
                nc.gpsimd.collective_compute(
                    "AllToAll",
                    mybir.AluOpType.bypass,
                    replica_groups=[list(range(CORES))],
                    ins=[dvi_t[h].opt()],
                    outs=[dvo.opt()],
                )
                a2a_out[h] = dvo

            ofull = [None, None]

            def fc_load(h):
                of = cpool.tile([P, hchunks, SROWS], FP8E4, tag=f"of{h}",
                                name=f"of{h}")
                nc.sync.dma_start(
                    of[:], a2a_out[h].rearrange("(o p) s -> p o s", p=P)
                )
                ofull[h] = of

            fcacc = cpool.tile([P, 8, 512], F32, tag="fcacc")
            eoall = cpool.tile([P, sb_blocks, DM], F32, tag="eoall")
            # [p, chunk-pair, slot, d] fp8 e4m3 (x32)
            wfc_sb = cpool.tile([P, hchunks, 2, DM], FP8E4, tag="wfc")
            qres_sb = cpool.tile([P, sb_blocks, DM], F32, tag="qre")
            tiles_fc = [(sb, nm) for sb in range(sb_blocks) for nm in range(DM // 512)]

            def fc_pass(h, ti, pf):
                """two DoubleRow matmuls accumulate head h's 4 chunks"""
                sb, nm = tiles_fc[ti]
                for a in range(2):
                    pp = 2 * h + a
                    nc.tensor.matmul(
                        pf[:],
                        ofull[h][:, 2 * a : 2 * a + 2, sb * P : (sb + 1) * P],
                        wfc_sb[:, pp, :, nm * 512 : (nm + 1) * 512],
                        start=(a == 0),
                        stop=(a == 1),
                        perf_mode=mybir.MatmulPerfMode.DoubleRow,
                    )

            def fc_passA(ti):
                pool = scps if ti % 2 == 0 else pjps
                pf = pool.tile([P, 512], F32, tag=pool is scps and "sc" or "pj",
                               name=f"pfA{ti}")
                fc_pass(0, ti, pf)
                if ti % 2 == 0:
                    nc.scalar.copy(out=fcacc[:, ti, :], in_=pf[:])
                else:
                    nc.vector.tensor_copy(out=fcacc[:, ti, :], in_=pf[:])

            def fc_passB_epi(ti):
                sb, nm = tiles_fc[ti]
                pool = scps if ti % 2 == 0 else pjps
                pf = pool.tile([P, 512], F32, tag=pool is scps and "sc" or "pj",
                               name=f"pfB{ti}")
                fc_pass(1, ti, pf)
                eo = eoall[:, sb, nm * 512 : (nm + 1) * 512]
                nc.vector.tensor_add(out=eo, in0=fcacc[:, ti, :], in1=pf[:])
                # undo the x16 (ship) * x32 (wfc) scaling inside the relu
                nc.scalar.activation(
                    out=eo, in_=eo, func=mybir.ActivationFunctionType.Relu,
                    scale=1.0 / 512.0,
                )
                nc.vector.tensor_add(
                    out=eo, in0=eo,
                    in1=qres_sb[:, sb, nm * 512 : (nm + 1) * 512],
                )

            # ================= schedule =================
            dvi_t[0] = dr.tile([CORES * DK, SROWS], FP8E4, name="a2advi0")
            dvi_t[1] = dr.tile([CORES * DK, SROWS], FP8E4, name="a2advi1")

            # phase A: q cols 0-2048 projected before sweep 1
            for g in range(4):
                qproj_group(g)

            def mk_xq_hook(g, eng):
                return lambda: load_xq(g, eng)

            def hook_late_consts():
                nc.gpsimd.dma_start(wfc_sb[:], wfcT_r[:])
                nc.gpsimd.dma_start(qres_sb[:], qres_r[:])

            # sweep 1: h0 qc0-3 with JIT k/v proj; stream remaining q chunks
            s1hooks = {
                6: mk_xq_hook(4, nc.gpsimd),
                12: mk_xq_hook(5, nc.sync),
                18: mk_xq_hook(6, nc.gpsimd),
                24: mk_xq_hook(7, nc.sync),
                28: hook_late_consts,
            }
            avsb_s1 = sweep(0, 0, True, hooks=s1hooks)

            # boundary: q cols 2048-4096 projections fill the PE while
            # sweep 1's reciprocal chain runs
            qproj_group(4)
            qproj_group(5)
            drain_stage2(0, 0, avsb_s1)
            qproj_group(6)
            qproj_group(7)

            # sweep 2: h0 qc4-7; its drain hooks into sweep 3
            avsb_s2 = sweep(0, 4, False)

            def hook_drain_s2():
                drain_stage2(0, 4, avsb_s2)
                ship_head(0)
                fc_load(0)

            # sweep 3: h1 qc0-3
            avsb_s3 = sweep(1, 0, False, hooks={3: hook_drain_s2})

            # boundary: h0 FC passes fill the PE while sweep 3's
            # reciprocal chain runs
            for ti in range(4):
                fc_passA(ti)
            drain_stage2(1, 0, avsb_s3)
            for ti in range(4, 8):
                fc_passA(ti)

            # sweep 4: h1 qc4-7
            avsb_s4 = sweep(1, 4, False)
            drain_stage2(1, 4, avsb_s4, split=True)
            ship_head(1)
            fc_load(1)

            # tail: h1 FC + epilogue; out stores batched into 2 DMAs
            for ti in range(4):
                fc_passB_epi(ti)
            nc.sync.dma_start(out_r[:, 0:2, :], eoall[:, 0:2, :])
            for ti in range(4, 8):
                fc_passB_epi(ti)
            nc.sync.dma_start(out_r[:, 2:4, :], eoall[:, 2:4, :])

    nc.compile()
    return nc


def _fc_perm():
    """Row permutation of WfcT matching the post-A2A dv order: FC lhsT chunk
    o (of head-h stream) partition p holds global dv row
    128*(2o + p//64) + h*64 + (p%64)."""
    perm = []
    for h in range(2):
        for o in range(4):
            for p in range(P):
                perm.append(128 * (2 * o + p // 64) + h * 64 + (p % 64))
    return np.array(perm)


def make_in_maps(q, k, v, Wq, Wk, Wv, Wfc):
    bf = ml_dtypes.bfloat16
    qT = np.ascontiguousarray(q.T).astype(bf)
    kT = np.ascontiguousarray(k.T).astype(bf)
    vT = np.ascontiguousarray(v.T).astype(bf)
    wfcT = np.ascontiguousarray(Wfc.T[_fc_perm()] * 32.0).astype(
        ml_dtypes.float8_e4m3)
    # selb[p, pair*128 + sub*64 + m] = 1 iff p == 32*(2*pair+sub): broadcasts
    # the reciprocal at partition 32*j to 64 output partitions
    # 16.0: normalized outputs ship x16 in fp8 e4m3
    selb = np.zeros((P, 2 * P), np.float32)
    for j in range(4):
        pair, sub = j // 2, j % 2
        selb[32 * j, pair * 128 + sub * 64 : pair * 128 + sub * 64 + 64] = 16.0
    selb = selb.astype(bf)
    in_maps = []
    for c in range(CORES):
        sl = slice(c * HL, (c + 1) * HL)
        in_maps.append(
            {
                "qT": qT,
                "kT": kT,
                "vT": vT,
                "wqT": np.ascontiguousarray(Wq[sl].T).astype(bf),
                "wkT": np.ascontiguousarray(Wk[sl].T).astype(bf),
                "wvT": np.ascontiguousarray(Wv[sl].T).astype(bf),
                "wfcT": wfcT,
                "selb": selb,
                "qres": np.ascontiguousarray(q[c * SROWS : (c + 1) * SROWS]).astype(
                    np.float32
                ),
            }
        )
    return in_maps


_NC_CACHE = {}


def kernel(q, k, v, Wq, Wk, Wv, Wfc):
    key = "full"
    if key not in _NC_CACHE:
        _NC_CACHE[key] = build()
    nc = _NC_CACHE[key]
    in_maps = make_in_maps(q, k, v, Wq, Wk, Wv, Wfc)
    trace = bool(int(os.environ.get("KERNEL_TRACE", "0")))
    tc_env = os.environ.get("KERNEL_TRACE_CORES", "")
    kw = {}
    if tc_env:
        kw["trace_cores"] = [int(x) for x in tc_env.split(",")]
    res = run_bass_kernel_spmd(nc, in_maps, list(range(CORES)), trace=trace, **kw)
    if trace:
        kernel.last_exec_time_ns = res.exec_time_ns
        kernel.last_profile = res
    out = np.concatenate([res.results[c]["out"] for c in range(CORES)], axis=0)
    return out.astype(np.float32)


# revision 41
# speedup vs baseline: 1.2358x; 1.2358x over previous
"""Multi-head attention (SEQ=4096, d_model=1024, 16 heads of d=64) on 8 TRN2
NeuronCores, tensor-parallel over heads (2 heads/core), per-head AllToAll to
sequence-parallel before the output projection.

v3 design (vs 480us baseline / 652us v2):
  - Block-outer attention sweeps with JIT projections: q/k/v are projected
    per column-group as their DMA chunks land, inside sweep 1, hiding the
    24MB input load and the projection PE work under attention.
  - AV matmuls run fp8 DoubleRow (K=256: two key blocks per matmul):
    vh2 [ks, pair, 2, 80] e4m3 (col 64 = ones -> softmax denominator),
    pt [ks, 2, qs] e5m2.  exp(s/8-0.5) alternates ScalarE LUT (e5m2 out)
    and VectorE Schraudolph uint8 bits = 0.72135*s + 50.62.
  - Batched normalization: the 4 sums rows of a sweep land at partitions
    {0,32,64,96} of one tile, ONE reciprocal, two selector-matmul
    broadcasts, DVE muls read the broadcast from PSUM.  The whole
    normalize tail is hooked into the NEXT sweep's PE stream so the PE
    never idles on the reciprocal latency (the v2 mistake: 16 single-
    partition reciprocals at 4us each on sweep boundaries).
  - Score pipeline depth 4 in sweeps 2-4 (sco tiles alternate two PSUM
    pools); FC head-0 passes fill the sweep3->4 boundary.
  - Outputs ship pre-normalized: one AllToAll per head, no sums A2A.
"""

import os
import sys

sys.path.insert(0, "/opt/trn_rl_repo")

import numpy as np
import ml_dtypes

import concourse.bass as bass
import concourse.mybir as mybir
import concourse.tile as tile
from concourse import bacc
from concourse.bass_utils import run_bass_kernel_spmd

SEQ = 4096
DM = 1024
NH = 16
DK = 64
DV = 64
CORES = 8
P = 128
HL = 2 * DK  # 128: two heads' head-dim per core
SROWS = SEQ // CORES  # 512 output rows per core
MO = DM // P  # 8 m-chunks of d_model
F32 = mybir.dt.float32
BF16 = mybir.dt.bfloat16
FP8E4 = mybir.dt.float8e4
FP8E5 = mybir.dt.float8e5
U8 = mybir.dt.uint8

AV_MODE = os.environ.get("AV_MODE", "fp8")  # fp8 (DoubleRow) | bf16 fallback

# exp(x/8 - SHIFT).  Real scores span +-75 (heavy tails): e5m2 Schraudolph
# bits = 0.72135*x + E5_B must stay in (0, 124) -> SHIFT=0.5: bits [2.9, 111].
ESHIFT = 0.5
# projections use x8-scaled fp8 weights -> raw scores are x64, vh is x8
WSCALE = 8.0
E5_A = 4.0 / (8.0 * WSCALE * WSCALE * np.log(2.0))
E5_B = 4.0 * (15.0 - ESHIFT / np.log(2.0)) - 0.172
BF_A = 128.0 / (8.0 * WSCALE * WSCALE * np.log(2.0))
BF_B = 16256.0 - 5.5 - 128.0 * ESHIFT / np.log(2.0)
EXP_SCALE = 0.125 / (WSCALE * WSCALE)


def build():
    kb = SEQ // P  # 32 key blocks
    npair = kb // 2  # 16 DoubleRow pairs
    qcw = 512
    sb_blocks = SROWS // P  # 4
    hchunks = (CORES * DK) // P  # 4 FC lhsT chunks per head
    fp8 = AV_MODE == "fp8"

    nc = bacc.Bacc(
        "TRN2",
        target_bir_lowering=False,
        debug=False,
        enable_asserts=True,
        num_devices=CORES,
    )

    qT = nc.dram_tensor("qT", [DM, SEQ], FP8E4, kind="ExternalInput").ap()
    kT = nc.dram_tensor("kT", [DM, SEQ], FP8E4, kind="ExternalInput").ap()
    vT = nc.dram_tensor("vT", [DM, SEQ], FP8E4, kind="ExternalInput").ap()
    wqT = nc.dram_tensor("wqT", [DM, HL], FP8E4, kind="ExternalInput").ap()
    wkT = nc.dram_tensor("wkT", [DM, HL], FP8E4, kind="ExternalInput").ap()
    wvT = nc.dram_tensor("wvT", [DM, HL], FP8E4, kind="ExternalInput").ap()
    # pre-permuted on host to match the post-A2A dv row order; fp8 e4m3
    # scaled x32 (outputs ship x16; the epilogue relu divides by 512)
    wfcT = nc.dram_tensor("wfcT", [DM, DM], FP8E4, kind="ExternalInput").ap()
    qres = nc.dram_tensor("qres", [SROWS, DM], F32, kind="ExternalInput").ap()
    # selector for the reciprocal broadcast (partition-1 writes are not
    # expressible on-device): selb[p, pair*128+sub*64+m] = (p==32*(2*pair+sub))
    selb_in = nc.dram_tensor("selb", [P, 2 * P], BF16, kind="ExternalInput").ap()
    out = nc.dram_tensor("out", [SROWS, DM], F32, kind="ExternalOutput").ap()

    qT_r = qT.rearrange("(o p) s -> p o s", p=P)
    kT_r = kT.rearrange("(o p) s -> p o s", p=P)
    vT_r = vT.rearrange("(o p) s -> p o s", p=P)
    wqT_r = wqT.rearrange("(o p) h -> p o h", p=P)
    wkT_r = wkT.rearrange("(o p) h -> p o h", p=P)
    wvT_r = wvT.rearrange("(o p) h -> p o h", p=P)
    # row (2*pp+s)*128+p -> [p, pp, s, d] for DoubleRow chunk pairs
    wfcT_r = wfcT.rearrange("(a s p) d -> p a s d", p=P, s=2)
    qres_r = qres.rearrange("(b p) d -> p b d", p=P)
    out_r = out.rearrange("(b p) d -> p b d", p=P)

    with tile.TileContext(nc) as tc:
        with (
            tc.tile_pool(name="const", bufs=1) as cpool,
            tc.tile_pool(name="xq", bufs=4) as xqpool,
            tc.tile_pool(name="xk", bufs=3) as xkpool,
            tc.tile_pool(name="xv", bufs=3) as xvpool,
            tc.tile_pool(name="pt", bufs=8) as ptpool,
            tc.tile_pool(name="avsb", bufs=2) as avsbpool,
            tc.tile_pool(name="eo", bufs=3) as eopool,
            tc.tile_pool(name="avps", bufs=4, space="PSUM") as avps,
            tc.tile_pool(name="scps", bufs=2, space="PSUM") as scps,
            tc.tile_pool(name="pjps", bufs=2, space="PSUM") as pjps,
            tc.tile_pool(name="dram", bufs=1, space="DRAM") as dr,
        ):
            # ---- first q chunks + weights (startup-critical order) ----
            xq = {}

            def load_xq(g, eng):
                t = xqpool.tile([P, MO, qcw], FP8E4, tag="xq", name=f"xq{g}")
                eng.dma_start(t[:], qT_r[:, :, g * qcw : (g + 1) * qcw])
                xq[g] = t

            wq_sb = cpool.tile([P, MO, HL], FP8E4, tag="wq")
            wk_sb = cpool.tile([P, MO, HL], FP8E4, tag="wk")
            wv_sb = cpool.tile([P, MO, HL], FP8E4, tag="wv")
            selb = cpool.tile([P, 2 * P], BF16, tag="selb")
            xks, xvs = {}, {}

            def load_kv(bg):
                xks[bg] = xkpool.tile([P, MO, qcw], FP8E4, tag="xk",
                                      name=f"xk{bg}")
                eng = nc.sync if bg % 2 == 0 else nc.gpsimd
                eng.dma_start(xks[bg][:], kT_r[:, :, bg * qcw : (bg + 1) * qcw])
                xvs[bg] = xvpool.tile([P, MO, qcw], FP8E4, tag="xv",
                                      name=f"xv{bg}")
                eng2 = nc.gpsimd if bg % 2 == 0 else nc.sync
                eng2.dma_start(xvs[bg][:], vT_r[:, :, bg * qcw : (bg + 1) * qcw])

            load_xq(0, nc.sync)
            nc.sync.dma_start(wq_sb[:], wqT_r[:])
            load_xq(1, nc.gpsimd)
            nc.gpsimd.dma_start(wk_sb[:], wkT_r[:])
            load_xq(2, nc.sync)
            load_xq(3, nc.gpsimd)
            nc.gpsimd.dma_start(wv_sb[:], wvT_r[:])
            nc.sync.dma_start(selb[:], selb_in[:])
            load_kv(0)
            load_kv(1)

            qhT2 = cpool.tile([P, SEQ], BF16, tag="qhT2")
            khT2z = [
                cpool.tile([P, SEQ], BF16, tag=f"khT2z{h}", name=f"khT2z{h}")
                for h in range(2)
            ]
            # ~1e-30 not 0.0: zero weights throttle the PE (activity gating)
            nc.vector.memset(khT2z[0][DK:P, :], 1e-30)
            nc.vector.memset(khT2z[1][0:DK, :], 1e-30)
            if fp8:
                # [ks, pair, slot, 80] e4m3; col 64 = ones (softmax denom),
                # cols 65-79 pad to a 16B DoubleRow weight stride
                vh2 = [
                    cpool.tile([P, npair, 2, 80], FP8E4, tag=f"vh{h}", name=f"vh{h}")
                    for h in range(2)
                ]
                for h in range(2):
                    nc.vector.memset(vh2[h][:, :, :, DV : DV + 1], 1.0)
                    nc.vector.memset(vh2[h][:, :, :, DV + 1 :], 0.002)
            else:
                vh2 = [
                    cpool.tile([P, kb, DV + 1], BF16, tag=f"vh{h}", name=f"vh{h}")
                    for h in range(2)
                ]
                for h in range(2):
                    nc.vector.memset(vh2[h][:, :, DV : DV + 1], 1.0)

            # normalized outputs (x16, fp8): qc j slab at partitions (j%2)*64,
            # cols j*512 (per head) -- ship DMAs slice per destination
            outT2 = [
                cpool.tile([P, SEQ], FP8E4, tag=f"outT2{h}", name=f"outT2{h}")
                for h in range(2)
            ]
            # sweep's 4 sums rows at partitions {0,32,64,96}; bf16 so the
            # selector-matmul broadcast runs as a fast bf16 matmul
            srow = cpool.tile([P, qcw], BF16, tag="srow")
            nc.vector.memset(srow[:], 1.0)
            negshift = cpool.tile([P, 1], F32, tag="negshift")
            nc.vector.memset(negshift[:], -float(ESHIFT))

            # ---- projection helpers ----
            def qproj_group(g):
                qp = pjps.tile([P, qcw], F32, tag="pj", name=f"qp{g}")
                for a in range(MO // 2):
                    nc.tensor.matmul(
                        qp[:],
                        wq_sb[:, 2 * a : 2 * a + 2, :],
                        xq[g][:, 2 * a : 2 * a + 2, :],
                        start=(a == 0),
                        stop=(a == MO // 2 - 1),
                        perf_mode=mybir.MatmulPerfMode.DoubleRow,
                    )
                nc.scalar.copy(out=qhT2[:, g * qcw : (g + 1) * qcw], in_=qp[:])

            def kproj_group(bg, xk):
                kp = pjps.tile([P, qcw], F32, tag="pj", name=f"kp{bg}")
                for a in range(MO // 2):
                    nc.tensor.matmul(
                        kp[:],
                        wk_sb[:, 2 * a : 2 * a + 2, :],
                        xk[:, 2 * a : 2 * a + 2, :],
                        start=(a == 0),
                        stop=(a == MO // 2 - 1),
                        perf_mode=mybir.MatmulPerfMode.DoubleRow,
                    )
                c0 = bg * qcw
                nc.scalar.copy(out=khT2z[0][0:DK, c0 : c0 + qcw], in_=kp[0:DK])
                nc.scalar.copy(out=khT2z[1][DK:HL, c0 : c0 + qcw], in_=kp[DK:HL])

            def vproj_block(b, xv, boff):
                pv = pjps.tile([P, HL], F32, tag="pj", name=f"pv{b}")
                for o in range(MO):
                    nc.tensor.matmul(
                        pv[:],
                        xv[:, o, boff : boff + P],
                        wv_sb[:, o, :],
                        start=(o == 0),
                        stop=(o == MO - 1),
                    )
                if fp8:
                    nc.vector.tensor_copy(
                        out=vh2[0][:, b // 2, b % 2, :DV], in_=pv[:, :DK]
                    )
                    nc.vector.tensor_copy(
                        out=vh2[1][:, b // 2, b % 2, :DV], in_=pv[:, DK:HL]
                    )
                else:
                    nc.vector.tensor_copy(out=vh2[0][:, b, :DV], in_=pv[:, :DK])
                    nc.vector.tensor_copy(out=vh2[1][:, b, :DV], in_=pv[:, DK:HL])

            exp_ctr = [0]

            def exp_tile(dst, dslot, sco):
                use_dve = exp_ctr[0] % 2 == 0
                exp_ctr[0] += 1
                if fp8:
                    if use_dve:
                        nc.vector.tensor_scalar(
                            out=dst[:, dslot, :].bitcast(U8),
                            in0=sco[:],
                            scalar1=float(E5_A),
                            scalar2=float(E5_B),
                            op0=mybir.AluOpType.mult,
                            op1=mybir.AluOpType.add,
                        )
                    else:
                        nc.scalar.activation(
                            out=dst[:, dslot, :],
                            in_=sco[:],
                            func=mybir.ActivationFunctionType.Exp,
                            scale=EXP_SCALE,
                            bias=negshift[:],
                        )
                else:
                    if use_dve:
                        nc.vector.tensor_scalar(
                            out=dst[:].bitcast(mybir.dt.int16),
                            in0=sco[:],
                            scalar1=float(BF_A),
                            scalar2=float(BF_B),
                            op0=mybir.AluOpType.mult,
                            op1=mybir.AluOpType.add,
                        )
                    else:
                        nc.scalar.activation(
                            out=dst[:],
                            in_=sco[:],
                            func=mybir.ActivationFunctionType.Exp,
                            scale=EXP_SCALE,
                            bias=negshift[:],
                        )

            # ---- batched drain machinery ----
            # stage 1 (at sweep end): free the avT banks fast -- ACT copies
            # the 64 dv rows to an SBUF slab, DVE copies the sums row into
            # srow at partition 32*j.
            # stage 2 (hooked into the NEXT sweep's PE stream): one
            # reciprocal over srow, two selector-matmul broadcasts, DVE
            # muls write normalized bf16 into outT2.
            def drain_stage1(h, jbase, avts):
                avsb = avsbpool.tile([P, 2, qcw], BF16, tag="avsb",
                                     name=f"avsb{h}_{jbase}")
                for j in range(4):
                    nc.scalar.copy(
                        out=avsb[(j % 2) * DK : (j % 2) * DK + DK, j // 2, :],
                        in_=avts[j][:DV, :],
                    )
                    nc.vector.tensor_copy(
                        out=srow[32 * j : 32 * j + 1, :],
                        in_=avts[j][DV : DV + 1, :],
                    )
                return avsb

            def drain_stage2(h, jbase, avsb, split=False):
                if not split:
                    with nc.allow_low_precision(reason="softmax denominators"):
                        nc.vector.reciprocal(srow[:], srow[:])
                for pair in range(2):
                    if split:
                        with nc.allow_low_precision(reason="softmax denominators"):
                            nc.vector.reciprocal(
                                srow[pair * DK : (pair + 1) * DK, :],
                                srow[pair * DK : (pair + 1) * DK, :],
                            )
                    bcp = pjps.tile([P, qcw], F32, tag="pj",
                                    name=f"bcp{h}_{jbase}_{pair}")
                    nc.tensor.matmul(
                        bcp[:],
                        selb[:, pair * P : (pair + 1) * P],
                        srow[:],
                        start=True,
                        stop=True,
                    )
                    for sub in range(2):
                        j = 2 * pair + sub
                        jq = jbase + j
                        p0 = (j % 2) * DK
                        nc.vector.tensor_mul(
                            out=outT2[h][p0 : p0 + DK, jq * qcw : (jq + 1) * qcw],
                            in0=avsb[p0 : p0 + DK, j // 2, :],
                            in1=bcp[p0 : p0 + DK, :],
                        )
                        ship_qc(h, jq)

            # dvi ship DMAs, one per destination core, emitted as soon as
            # that qc's normalized slab exists
            dvi_t = [None, None]

            def ship_qc(h, jq):
                eng = nc.sync if jq % 2 == 0 else nc.gpsimd
                eng.dma_start(
                    dvi_t[h][jq * DK : (jq + 1) * DK, :],
                    outT2[h][(jq % 2) * DK : (jq % 2) * DK + DK,
                             jq * qcw : (jq + 1) * qcw],
                )

            # ---- attention sweep ----
            def sweep(h, jbase, with_proj, hooks=None):
                avts = [
                    avps.tile([80 if fp8 else DV + 1, qcw], F32, tag="av",
                              name=f"av{h}_{jbase + j}")
                    for j in range(4)
                ]
                pts = {}
                deep = not with_proj
                for b in range(kb):
                    if with_proj:
                        bg = b // 4
                        if b % 4 == 0:
                            if bg + 2 < kb // 4:
                                load_kv(bg + 2)
                            kproj_group(bg, xks[bg])
                        vproj_block(b, xvs[bg], (b % 4) * P)
                    for j in range(4):
                        jq = jbase + j
                        q0 = jq * qcw
                        # depth-4 score pipeline outside sweep 1
                        pool = pjps if (deep and (b * 4 + j) % 2 == 1) else scps
                        sco = pool.tile([P, qcw], F32, tag=pool is scps and "sc" or "pj",
                                        name=f"sc{b}_{j}")
                        nc.tensor.matmul(
                            sco[:],
                            khT2z[h][:, b * P : (b + 1) * P],
                            qhT2[:, q0 : q0 + qcw],
                            start=True,
                            stop=True,
                        )
                        if fp8:
                            if b % 2 == 0:
                                pts[j] = ptpool.tile([P, 2, qcw], FP8E5, tag="pt",
                                                     name=f"pt{b}_{j}")
                            exp_tile(pts[j], b % 2, sco)
                            if b % 2 == 1:
                                nc.tensor.matmul(
                                    avts[j][:],
                                    vh2[h][:, b // 2, :, :],
                                    pts[j][:],
                                    start=(b == 1),
                                    stop=(b == kb - 1),
                                    perf_mode=mybir.MatmulPerfMode.DoubleRow,
                                )
                        else:
                            ptb = ptpool.tile([P, qcw], BF16, tag="pt",
                                              name=f"pt{b}_{j}")
                            exp_tile(ptb, 0, sco)
                            nc.tensor.matmul(
                                avts[j][:],
                                vh2[h][:, b, :],
                                ptb[:],
                                start=(b == 0),
                                stop=(b == kb - 1),
                            )
                    if hooks and b in hooks:
                        hooks[b]()
                return drain_stage1(h, jbase, avts)

            # ---- AllToAll of normalized dv rows ----
            a2a_out = [None, None]

            def ship_head(h):
                dvo = dr.tile([CORES * DK, SROWS], FP8E4, name=f"a2advo{h}")
                nc.gpsimd.collective_compute(
                    "AllToAll",
                    mybir.AluOpType.bypass,
                    replica_groups=[list(range(CORES))],
                    ins=[dvi_t[h].opt()],
                    outs=[dvo.opt()],
                )
                a2a_out[h] = dvo

            ofull = [None, None]

            def fc_load(h):
                of = cpool.tile([P, hchunks, SROWS], FP8E4, tag=f"of{h}",
                                name=f"of{h}")
                nc.sync.dma_start(
                    of[:], a2a_out[h].rearrange("(o p) s -> p o s", p=P)
                )
                ofull[h] = of

            fcacc = cpool.tile([P, 8, 512], F32, tag="fcacc")
            eoall = cpool.tile([P, sb_blocks, DM], F32, tag="eoall")
            # [p, chunk-pair, slot, d] fp8 e4m3 (x32)
            wfc_sb = cpool.tile([P, hchunks, 2, DM], FP8E4, tag="wfc")
            qres_sb = cpool.tile([P, sb_blocks, DM], F32, tag="qre")
            tiles_fc = [(sb, nm) for sb in range(sb_blocks) for nm in range(DM // 512)]

            def fc_pass(h, ti, pf):
                """two DoubleRow matmuls accumulate head h's 4 chunks"""
                sb, nm = tiles_fc[ti]
                for a in range(2):
                    pp = 2 * h + a
                    nc.tensor.matmul(
                        pf[:],
                        ofull[h][:, 2 * a : 2 * a + 2, sb * P : (sb + 1) * P],
                        wfc_sb[:, pp, :, nm * 512 : (nm + 1) * 512],
                        start=(a == 0),
                        stop=(a == 1),
                        perf_mode=mybir.MatmulPerfMode.DoubleRow,
                    )

            def fc_passA(ti):
                pool = scps if ti % 2 == 0 else pjps
                pf = pool.tile([P, 512], F32, tag=pool is scps and "sc" or "pj",
                               name=f"pfA{ti}")
                fc_pass(0, ti, pf)
                if ti % 2 == 0:
                    nc.scalar.copy(out=fcacc[:, ti, :], in_=pf[:])
                else:
                    nc.vector.tensor_copy(out=fcacc[:, ti, :], in_=pf[:])

            def fc_passB_epi(ti):
                sb, nm = tiles_fc[ti]
                pool = scps if ti % 2 == 0 else pjps
                pf = pool.tile([P, 512], F32, tag=pool is scps and "sc" or "pj",
                               name=f"pfB{ti}")
                fc_pass(1, ti, pf)
                eo = eoall[:, sb, nm * 512 : (nm + 1) * 512]
                nc.vector.tensor_add(out=eo, in0=fcacc[:, ti, :], in1=pf[:])
                # undo the x16 (ship) * x32 (wfc) scaling inside the relu
                nc.scalar.activation(
                    out=eo, in_=eo, func=mybir.ActivationFunctionType.Relu,
                    scale=1.0 / 512.0,
                )
                nc.vector.tensor_add(
                    out=eo, in0=eo,
                    in1=qres_sb[:, sb, nm * 512 : (nm + 1) * 512],
                )

            # ================= schedule =================
            dvi_t[0] = dr.tile([CORES * DK, SROWS], FP8E4, name="a2advi0")
            dvi_t[1] = dr.tile([CORES * DK, SROWS], FP8E4, name="a2advi1")

            # phase A: q cols 0-2048 projected before sweep 1
            for g in range(4):
                qproj_group(g)

            def mk_xq_hook(g, eng):
                return lambda: load_xq(g, eng)

            def hook_late_consts():
                nc.gpsimd.dma_start(wfc_sb[:], wfcT_r[:])
                nc.gpsimd.dma_start(qres_sb[:], qres_r[:])

            # sweep 1: h0 qc0-3 with JIT k/v proj; stream remaining q chunks
            s1hooks = {
                6: mk_xq_hook(4, nc.gpsimd),
                12: mk_xq_hook(5, nc.sync),
                18: mk_xq_hook(6, nc.gpsimd),
                24: mk_xq_hook(7, nc.sync),
                28: hook_late_consts,
            }
            avsb_s1 = sweep(0, 0, True, hooks=s1hooks)

            # boundary: q cols 2048-4096 projections fill the PE while
            # sweep 1's reciprocal chain runs
            qproj_group(4)
            qproj_group(5)
            drain_stage2(0, 0, avsb_s1)
            qproj_group(6)
            qproj_group(7)

            # sweep 2: h0 qc4-7; its drain hooks into sweep 3
            avsb_s2 = sweep(0, 4, False)

            def hook_drain_s2():
                drain_stage2(0, 4, avsb_s2)
                ship_head(0)
                fc_load(0)

            # sweep 3: h1 qc0-3
            avsb_s3 = sweep(1, 0, False, hooks={3: hook_drain_s2})

            # boundary: h0 FC passes fill the PE while sweep 3's
            # reciprocal chain runs
            for ti in range(4):
                fc_passA(ti)
            drain_stage2(1, 0, avsb_s3)
            for ti in range(4, 8):
                fc_passA(ti)

            # sweep 4: h1 qc4-7
            avsb_s4 = sweep(1, 4, False)
            drain_stage2(1, 4, avsb_s4, split=True)
            ship_head(1)
            fc_load(1)

            # tail: h1 FC + epilogue; out stores batched into 2 DMAs
            for ti in range(4):
                fc_passB_epi(ti)
            nc.sync.dma_start(out_r[:, 0:2, :], eoall[:, 0:2, :])
            for ti in range(4, 8):
                fc_passB_epi(ti)
            nc.sync.dma_start(out_r[:, 2:4, :], eoall[:, 2:4, :])

    nc.compile()
    return nc


def _fc_perm():
    """Row permutation of WfcT matching the post-A2A dv order: FC lhsT chunk
    o (of head-h stream) partition p holds global dv row
    128*(2o + p//64) + h*64 + (p%64)."""
    perm = []
    for h in range(2):
        for o in range(4):
            for p in range(P):
                perm.append(128 * (2 * o + p // 64) + h * 64 + (p % 64))
    return np.array(perm)


def make_in_maps(q, k, v, Wq, Wk, Wv, Wfc):
    bf = ml_dtypes.bfloat16
    e4 = ml_dtypes.float8_e4m3
    qT = np.ascontiguousarray(q.T).astype(e4)
    kT = np.ascontiguousarray(k.T).astype(e4)
    vT = np.ascontiguousarray(v.T).astype(e4)
    wfcT = np.ascontiguousarray(Wfc.T[_fc_perm()] * 32.0).astype(
        ml_dtypes.float8_e4m3)
    # selb[p, pair*128 + sub*64 + m] = 1 iff p == 32*(2*pair+sub): broadcasts
    # the reciprocal at partition 32*j to 64 output partitions
    # 16.0: normalized outputs ship x16 in fp8 e4m3
    selb = np.zeros((P, 2 * P), np.float32)
    for j in range(4):
        pair, sub = j // 2, j % 2
        selb[32 * j, pair * 128 + sub * 64 : pair * 128 + sub * 64 + 64] = 2.0
    selb = selb.astype(bf)
    in_maps = []
    for c in range(CORES):
        sl = slice(c * HL, (c + 1) * HL)
        in_maps.append(
            {
                "qT": qT,
                "kT": kT,
                "vT": vT,
                "wqT": np.ascontiguousarray(Wq[sl].T * 8.0).astype(e4),
                "wkT": np.ascontiguousarray(Wk[sl].T * 8.0).astype(e4),
                "wvT": np.ascontiguousarray(Wv[sl].T * 8.0).astype(e4),
                "wfcT": wfcT,
                "selb": selb,
                "qres": np.ascontiguousarray(q[c * SROWS : (c + 1) * SROWS]).astype(
                    np.float32
                ),
            }
        )
    return in_maps


_NC_CACHE = {}


def kernel(q, k, v, Wq, Wk, Wv, Wfc):
    key = "full"
    if key not in _NC_CACHE:
        _NC_CACHE[key] = build()
    nc = _NC_CACHE[key]
    in_maps = make_in_maps(q, k, v, Wq, Wk, Wv, Wfc)
    trace = bool(int(os.environ.get("KERNEL_TRACE", "0")))
    tc_env = os.environ.get("KERNEL_TRACE_CORES", "")
    kw = {}
    if tc_env:
        kw["trace_cores"] = [int(x) for x in tc_env.split(",")]
    res = run_bass_kernel_spmd(nc, in_maps, list(range(CORES)), trace=trace, **kw)
    if trace:
        kernel.last_exec_time_ns = res.exec_time_ns
        kernel.last_profile = res
    out = np.concatenate([res.results[c]["out"] for c in range(CORES)], axis=0)
    return out.astype(np.float32)
